# revision 1
# baseline (speedup 1.0000x reference)
"""Multi-head attention (no softmax) on 8 trn2 NeuronCores.

Reference: out = ((x @ Wqkv.T -> q,k,v per head) ; (q @ k.T * s) @ v ; concat ; @ Wproj.T)

Because there is no softmax the attention is linear:
    (q @ k.T) @ v == q @ (k.T @ v),  k.T @ v is only 64x64 per head,
so the T x T score matrices never need to exist. Per head:
    M_h = (s * k_h).T @ v_h        (64 x 64, reduced over ALL tokens of the batch)
    out += (q_h @ M_h) @ Wproj_h.T

Sharding: token-parallel. Core c owns batch b=c//2, token half c%2 (512 tokens).
M_h needs a reduction over the full batch -> two tiny 128KB AllGathers between
the two cores of each batch (pipelined, peer-add done locally on DVE),
overlapped with the second kv half, the q matmuls, and the first half of the
output projection (which only needs heads 0-7).

All matmuls run in float32r (full PE rate; fp32 is 4x slower). Inputs are
pre-rounded to fp32r on the host (matmul is then exact), intermediates are
rounded by the PSUM->SBUF eviction copies. The head-dim scale 1/8 is folded
into W_k on the host (exact, power of two).

Weights are fed pre-transposed/pre-permuted so every matmul operand has the
contraction dim on partitions with unit-stride DMAs:
  wqkvT (E, 3E): cols 0:E = q features grouped h*64+j, E:2E = k (scaled), 2E:3E = v
  wpT   (E, E):  wpT[f, o] = W_proj[o, f]
  xT_c  (E, 512) per core.

DMA triggers: Sync queue carries x and weights in program order (paces the kv
phase); the GpSimd queue carries the collective bounces so the gathers fire
the moment their inputs are ready; output stores alternate between the two.
"""

import numpy as np

B, T, E = 4, 1024, 1024
NH, HD = 16, 64
N_CORES = 8
TPC = T // 2  # tokens per core = 512

_built = None


def _round_fp32r(a: np.ndarray) -> np.ndarray:
    """Round fp32 to fp32r (11 explicit mantissa bits, RNE) — matches HW."""
    u = np.ascontiguousarray(a, dtype=np.float32).view(np.uint32).astype(np.uint64)
    u = u + 0x7FF + ((u >> 12) & 1)
    u = (u & ~np.uint64(0xFFF)).astype(np.uint32)
    return u.view(np.float32).reshape(a.shape)


def _build():
    """Build + compile the 8-core SPMD Bass program once."""
    global _built
    if _built is not None:
        return _built

    import concourse.mybir as mybir
    import concourse.tile as tile
    from concourse import bacc

    f32 = mybir.dt.float32
    f32r = mybir.dt.float32r
    GROUPS = [[0, 1], [2, 3], [4, 5], [6, 7]]

    nc = bacc.Bacc("TRN2", target_bir_lowering=False, debug=False, num_devices=N_CORES)
    xT = nc.dram_tensor("xT", [E, TPC], f32r, kind="ExternalInput").ap()
    wqkvT = nc.dram_tensor("wqkvT", [E, 3 * E], f32r, kind="ExternalInput").ap()
    wpT = nc.dram_tensor("wpT", [E, E], f32r, kind="ExternalInput").ap()
    out = nc.dram_tensor("out", [TPC, E], f32, kind="ExternalOutput").ap()

    def evict(i, dst, src):
        # spread PSUM->SBUF eviction copies across DVE and ACT
        if i % 2 == 0:
            nc.vector.tensor_copy(dst, src)
        else:
            nc.scalar.copy(dst, src)

    with tile.TileContext(nc) as tc:
        with (
            tc.tile_pool(name="xp", bufs=1) as xp,
            tc.tile_pool(name="wkvp", bufs=4) as wkvp,
            tc.tile_pool(name="kvp", bufs=1) as kvp,
            tc.tile_pool(name="wqp", bufs=2) as wqp,
            tc.tile_pool(name="wpp", bufs=1) as wpp,
            tc.tile_pool(name="qp", bufs=1) as qp,
            tc.tile_pool(name="mres", bufs=1) as mres,
            tc.tile_pool(name="op", bufs=2) as op,
            tc.tile_pool(name="dram", bufs=1, space="DRAM") as dram,
            tc.tile_pool(name="psA", bufs=4, space="PSUM") as psA,
            tc.tile_pool(name="psM", bufs=2, space="PSUM") as psM,
        ):
            # ---- input DMAs ----
            # kv fc-group order: k half 0, v half 0, k half 1, v half 1 so the
            # first half of the M blocks is ready after two groups.
            FC_ORDER = [0, 2, 1, 3]
            xsb = []
            wkv_groups = {}
            for e in range(8):
                t = xp.tile([128, TPC], f32r, tag=f"x{e}")
                nc.sync.dma_start(t[:], xT[128 * e:128 * (e + 1), :])
                xsb.append(t)
                fc = FC_ORDER[0]
                w = wkvp.tile([128, 512], f32r, tag=f"wkv{e}", name=f"wkv{fc}_{e}")
                nc.sync.dma_start(
                    w[:], wqkvT[128 * e:128 * (e + 1), E + 512 * fc:E + 512 * (fc + 1)])
                wkv_groups.setdefault(fc, []).append(w)
            for fc in FC_ORDER[1:]:
                for e in range(8):
                    w = wkvp.tile([128, 512], f32r, tag=f"wkv{e}", name=f"wkv{fc}_{e}")
                    nc.sync.dma_start(
                        w[:], wqkvT[128 * e:128 * (e + 1), E + 512 * fc:E + 512 * (fc + 1)])
                    wkv_groups.setdefault(fc, []).append(w)

            # q weights on the Sync queue AFTER the kv weights so their
            # transfers don't steal HBM bandwidth from the critical kv stream;
            # streamed as two half-column chunk groups
            wq_groups = []
            for h in range(2):
                grp = []
                for e in range(8):
                    t = wqp.tile([128, 512], f32r, tag=f"wq{e}", name=f"wq{h}_{e}")
                    nc.sync.dma_start(
                        t[:], wqkvT[128 * e:128 * (e + 1), 512 * h:512 * (h + 1)])
                    grp.append(t)
                wq_groups.append(grp)

            kvsb = [kvp.tile([128, 2 * E], f32r, tag=f"kv{tt}", name=f"kv{tt}")
                    for tt in range(4)]
            Mbd = mres.tile([128, 1024], f32r, tag="Mbd")
            nc.gpsimd.memset(Mbd[:].bitcast(f32), 0.0)

            bout = [None, None]

            def kv_quarter(fc):
                i = 0
                for tt in range(4):
                    ps = psA.tile([128, 512], f32, tag="big")
                    for e in range(8):
                        nc.tensor.matmul(
                            ps[:],
                            xsb[e][:, 128 * tt:128 * (tt + 1)],
                            wkv_groups[fc][e][:],
                            start=(e == 0), stop=(e == 7),
                        )
                    evict(i, kvsb[tt][:, 512 * fc:512 * (fc + 1)], ps[:])
                    i += 1

            def m_half(g):
                # M blocks 4g..4g+3 from k cols [512g:512g+512], v cols
                # [E+512g : E+512g+512]; keep only diagonal 64x64 sub-blocks.
                mp = psM.tile([128, 512], f32, tag="mp", name=f"mp{g}")
                for j in range(4):
                    blk = 4 * g + j
                    for tt in range(4):
                        nc.tensor.matmul(
                            mp[:, 128 * j:128 * (j + 1)],
                            kvsb[tt][:, 128 * blk:128 * (blk + 1)],
                            kvsb[tt][:, E + 128 * blk:E + 128 * (blk + 1)],
                            start=(tt == 0), stop=(tt == 3),
                        )
                Msb = mres.tile([128, 256], f32, tag=f"Msb{g}", name=f"Msb{g}")
                for j in range(4):
                    nc.vector.tensor_copy(Msb[0:64, 64 * j:64 * j + 64],
                                          mp[0:64, 128 * j:128 * j + 64])
                    nc.vector.tensor_copy(Msb[64:128, 64 * j:64 * j + 64],
                                          mp[64:128, 128 * j + 64:128 * (j + 1)])
                # bounce to DRAM, two DMAs so the transfers ride parallel HW queues
                bin_ = dram.tile([128, 256], f32, name=f"bin{g}")
                bo = dram.tile([256, 256], f32, name=f"bout{g}")
                nc.gpsimd.dma_start(bin_[0:64, :], Msb[0:64, :])
                nc.gpsimd.dma_start(bin_[64:128, :], Msb[64:128, :])
                nc.gpsimd.collective_compute(
                    "AllGather", mybir.AluOpType.bypass, replica_groups=GROUPS,
                    ins=[bin_.opt()], outs=[bo.opt()],
                )
                MrA = mres.tile([128, 256], f32, tag=f"MrA{g}", name=f"MrA{g}")
                MrB = mres.tile([128, 256], f32, tag=f"MrB{g}", name=f"MrB{g}")
                nc.gpsimd.dma_start(MrA[0:64, :], bo[0:64, :])
                nc.gpsimd.dma_start(MrA[64:128, :], bo[64:128, :])
                nc.sync.dma_start(MrB[0:64, :], bo[128:192, :])
                nc.sync.dma_start(MrB[64:128, :], bo[192:256, :])
                bout[g] = (MrA, MrB)

            def m_post(g):
                # add both ranks' partials straight into Mbd diagonal spots
                MrA, MrB = bout[g]
                for j in range(4):
                    blk = 4 * g + j
                    nc.vector.tensor_add(
                        Mbd[0:64, 128 * blk:128 * blk + 64],
                        MrA[0:64, 64 * j:64 * j + 64],
                        MrB[0:64, 64 * j:64 * j + 64])
                    nc.vector.tensor_add(
                        Mbd[64:128, 128 * blk + 64:128 * (blk + 1)],
                        MrA[64:128, 64 * j:64 * j + 64],
                        MrB[64:128, 64 * j:64 * j + 64])

            # ---- kv + M + gathers, pipelined in halves ----
            kv_quarter(0)      # k cols 0:512
            kv_quarter(2)      # v cols 0:512
            m_half(0)          # M blocks 0-3 + AllGather #1 (in flight)
            kv_quarter(1)      # k cols 512:1024
            kv_quarter(3)      # v cols 512:1024
            m_half(1)          # M blocks 4-7 + AllGather #2 (in flight)

            wp = []
            for f in range(8):
                t = wpp.tile([128, E], f32r, tag=f"wp{f}")
                nc.sync.dma_start(t[:], wpT[128 * f:128 * (f + 1), :])
                wp.append(t)

            # ---- q (feature-major qT, (1024f, 512t)), overlaps the gathers ----
            qsb = [qp.tile([128, TPC], f32r, tag=f"q{f}", name=f"q{f}")
                   for f in range(8)]
            for fq in range(8):
                wqg = wq_groups[fq // 4]
                ps = psA.tile([128, 512], f32, tag="big")
                for e in range(8):
                    nc.tensor.matmul(
                        ps[:],
                        wqg[e][:, 128 * (fq % 4):128 * (fq % 4 + 1)],
                        xsb[e][:],
                        start=(e == 0), stop=(e == 7),
                    )
                evict(fq, qsb[fq][:], ps[:])

            m_post(0)
            m_post(1)

            # ---- att: attT_blk = Mbd_blk.T @ qT_blk (in-place into q tiles) ----
            for blk in range(8):
                ps = psA.tile([128, 512], f32, tag="big")
                nc.tensor.matmul(ps[:], Mbd[:, 128 * blk:128 * (blk + 1)],
                                 qsb[blk][:], start=True, stop=True)
                evict(blk, qsb[blk][:], ps[:])
            attsb = qsb

            # ---- out = attT.T @ wpT  ((512t, 1024o)) ----
            i = 0
            for tt in range(4):
                for oc in range(2):
                    ps = psA.tile([128, 512], f32, tag="big")
                    for f in range(8):
                        nc.tensor.matmul(
                            ps[:],
                            attsb[f][:, 128 * tt:128 * (tt + 1)],
                            wp[f][:, 512 * oc:512 * (oc + 1)],
                            start=(f == 0), stop=(f == 7),
                        )
                    ot = op.tile([128, 512], f32, tag="osb")
                    evict(i, ot[:], ps[:])
                    eng = nc.sync if i % 2 else nc.gpsimd
                    i += 1
                    eng.dma_start(
                        out[128 * tt:128 * (tt + 1), 512 * oc:512 * (oc + 1)],
                        ot[:],
                    )

    nc.compile()
    _built = nc
    return nc


LAST_RESULTS = None  # BassKernelResults of the most recent kernel() call


def kernel(x: np.ndarray, W_qkv: np.ndarray, W_proj: np.ndarray) -> np.ndarray:
    global LAST_RESULTS
    from concourse import bass_utils

    nc = _build()

    x = np.ascontiguousarray(x, dtype=np.float32)
    W_qkv = np.ascontiguousarray(W_qkv, dtype=np.float32)
    W_proj = np.ascontiguousarray(W_proj, dtype=np.float32)

    # head-grouping permutation: grouped feature h*64+j <- original row j*16+h
    perm = np.arange(E).reshape(HD, NH).T.ravel()
    Wq_g = W_qkv[perm]
    Wk_g = W_qkv[E + perm] * np.float32(HD ** -0.5)  # exact: 1/8
    Wv_g = W_qkv[2 * E + perm]
    wqkvT_np = _round_fp32r(np.concatenate([Wq_g, Wk_g, Wv_g], 0).T)
    wpT_np = _round_fp32r(W_proj.T)

    in_maps = []
    for c in range(N_CORES):
        b, half = c // 2, c % 2
        xT_c = _round_fp32r(x[b, half * TPC:(half + 1) * TPC, :].T)
        in_maps.append({"xT": xT_c, "wqkvT": wqkvT_np, "wpT": wpT_np})

    import os as _os
    _tc = _os.environ.get("KERNEL_TRACE_CORES")
    _kw = {"trace_cores": [int(x) for x in _tc.split(",")]} if _tc else {}
    res = bass_utils.run_bass_kernel_spmd(nc, in_maps, core_ids=list(range(N_CORES)), **_kw)
    LAST_RESULTS = res

    out = np.empty((B, T, E), dtype=np.float32)
    for c in range(N_CORES):
        b, half = c // 2, c % 2
        out[b, half * TPC:(half + 1) * TPC, :] = res.results[c]["out"]
    return out



# revision 3
# speedup vs baseline: 1.2460x; 1.2460x over previous
"""Multi-head attention (no softmax) on 8 trn2 NeuronCores.

Reference: out = ((x @ Wqkv.T -> q,k,v per head) ; (q @ k.T * s) @ v ; concat ; @ Wproj.T)

No softmax -> attention is linear:
    (q @ k.T) @ v == q @ (k.T @ v),  k.T @ v is 64x64 per head,
so the T x T score matrices never exist. Per head:
    M_h = (s * k_h).T @ v_h        (64 x 64, reduced over ALL tokens of the batch)
    out += (q_h @ M_h) @ Wproj_h.T

Sharding: token-parallel. Core c owns batch b=c//2, token half c%2 (512 tokens).
M_h needs the full batch -> two tiny 64KB bf16 AllGathers between the pair
cores, pipelined behind the second kv half and the q phase.

Everything is bf16 (inputs pre-rounded on host; PSUM accumulation is f32;
PSUM->SBUF evictions cast to bf16). This halves both HBM traffic (the
mid-kernel is DMA-paced) and SBUF footprint vs f32r, at the same PE rate.
Measured end-to-end rel err ~5e-3.

Scheduling notes (from the v1 trace):
 - kv loops are e-outer so each wkv-tile DMA semaphore gates 8 matmuls, not
   1: LDWEIGHTS then hides behind the previous matmul stream (427ns -> ~250ns
   per 512-col matmul).
 - DMA descriptors are 2-8KB: x rides one [128, 4096] tile (8KB rows),
   wkv as 8 [128, 2048] tiles (4KB rows), wq/wp as [128, 1024] (2KB rows).
 - The M-partial AllGathers trigger at ~34us and ~54us (v1: 47/77us), so the
   collective latency hides under the q phase.
 - The 1/8 head scale is folded into W_k host-side (exact in bf16).
"""

import numpy as np

B, T, E = 4, 1024, 1024
NH, HD = 16, 64
N_CORES = 8
TPC = T // 2  # tokens per core = 512

_built = None


def _round_bf16(a: np.ndarray) -> np.ndarray:
    """Round fp32 to bf16 (RNE) as an ml_dtypes.bfloat16 array."""
    import ml_dtypes
    return np.ascontiguousarray(a, dtype=np.float32).astype(ml_dtypes.bfloat16)


def _build():
    """Build + compile the 8-core SPMD Bass program once."""
    global _built
    if _built is not None:
        return _built

    import concourse.mybir as mybir
    import concourse.tile as tile
    from concourse import bacc

    f32 = mybir.dt.float32
    bf16 = mybir.dt.bfloat16
    GROUPS = [[0, 1], [2, 3], [4, 5], [6, 7]]

    nc = bacc.Bacc("TRN2", target_bir_lowering=False, debug=False, num_devices=N_CORES)
    # xTi[p, 512e + t] = x[core tokens][t, 128e + p]  (8KB rows)
    xTi = nc.dram_tensor("xTi", [128, 8 * TPC], bf16, kind="ExternalInput").ap()
    # wkvT rows = in-features, cols = [k h0 | v h0 | k h1 | v h1] (grouped, 4KB rows)
    wkvT = nc.dram_tensor("wkvT", [E, 2 * E], bf16, kind="ExternalInput").ap()
    wqT = nc.dram_tensor("wqT", [E, E], bf16, kind="ExternalInput").ap()
    wpT = nc.dram_tensor("wpT", [E, E], bf16, kind="ExternalInput").ap()
    out = nc.dram_tensor("out", [TPC, E], f32, kind="ExternalOutput").ap()

    ecnt = [0]

    def evict(dst, src):
        # spread PSUM->SBUF eviction casts across DVE and ACT
        if ecnt[0] % 2 == 0:
            nc.vector.tensor_copy(dst, src)
        else:
            nc.scalar.copy(dst, src)
        ecnt[0] += 1

    with tile.TileContext(nc) as tc:
        with (
            tc.tile_pool(name="xp", bufs=1) as xp,
            tc.tile_pool(name="wkvp", bufs=1) as wkvp,
            tc.tile_pool(name="wqp", bufs=1) as wqp,
            tc.tile_pool(name="wpp", bufs=1) as wpp,
            tc.tile_pool(name="kvp", bufs=1) as kvp,
            tc.tile_pool(name="qp", bufs=1) as qp,
            tc.tile_pool(name="mres", bufs=1) as mres,
            tc.tile_pool(name="op", bufs=2) as op,
            tc.tile_pool(name="dram", bufs=1, space="DRAM") as dram,
            tc.tile_pool(name="ps", bufs=1, space="PSUM") as psp,
        ):
            # ---- PSUM: 8 tags x [128,512] f32 (1 bank each), round-robin ----
            pcnt = [0]

            def ps_alloc():
                t = psp.tile([128, 512], f32, tag=f"ps{pcnt[0] % 8}", name="ps")
                pcnt[0] += 1
                return t

            # ---- input DMAs (sync queue, program order = stream order) ----
            xsb = xp.tile([128, 8 * TPC], bf16, tag="x")
            nc.sync.dma_start(xsb[:], xTi[:, :])
            wkv = []
            for e in range(8):
                w = wkvp.tile([128, 2 * E], bf16, tag=f"wkv{e}", name=f"wkv{e}")
                nc.sync.dma_start(w[:], wkvT[128 * e:128 * (e + 1), :])
                wkv.append(w)
            wq = []
            for e in range(8):
                w = wqp.tile([128, E], bf16, tag=f"wq{e}", name=f"wq{e}")
                nc.sync.dma_start(w[:], wqT[128 * e:128 * (e + 1), :])
                wq.append(w)
            wp = []
            for f in range(8):
                w = wpp.tile([128, E], bf16, tag=f"wp{f}", name=f"wp{f}")
                nc.sync.dma_start(w[:], wpT[128 * f:128 * (f + 1), :])
                wp.append(w)

            kvsb = [kvp.tile([128, 2 * E], bf16, tag=f"kv{tt}", name=f"kv{tt}")
                    for tt in range(4)]
            qsb = [qp.tile([128, TPC], bf16, tag=f"q{f}", name=f"q{f}")
                   for f in range(8)]
            Mbd = mres.tile([128, E], bf16, tag="Mbd")
            nc.gpsimd.memset(Mbd[:].bitcast(f32), 0.0)

            gath = [None, None]

            def kv_half(h, tt_order, e_outer):
                # kv cols [1024h : 1024h+1024] = [k half h | v half h]
                ps = {}
                if e_outer:
                    for tt in tt_order:
                        for cg in range(2):
                            ps[tt, cg] = ps_alloc()
                    for e in range(8):
                        for tt in tt_order:
                            for cg in range(2):
                                nc.tensor.matmul(
                                    ps[tt, cg][:],
                                    xsb[:, 512 * e + 128 * tt:512 * e + 128 * (tt + 1)],
                                    wkv[e][:, 1024 * h + 512 * cg:1024 * h + 512 * (cg + 1)],
                                    start=(e == 0), stop=(e == 7),
                                )
                    for tt in tt_order:
                        for cg in range(2):
                            evict(kvsb[tt][:, 1024 * h + 512 * cg:1024 * h + 512 * (cg + 1)],
                                  ps[tt, cg][:])
                else:
                    for tt in tt_order:
                        for cg in range(2):
                            p = ps_alloc()
                            for e in range(8):
                                nc.tensor.matmul(
                                    p[:],
                                    xsb[:, 512 * e + 128 * tt:512 * e + 128 * (tt + 1)],
                                    wkv[e][:, 1024 * h + 512 * cg:1024 * h + 512 * (cg + 1)],
                                    start=(e == 0), stop=(e == 7),
                                )
                            evict(kvsb[tt][:, 1024 * h + 512 * cg:1024 * h + 512 * (cg + 1)],
                                  p[:])

            def m_half(g):
                # M blocks 4g..4g+3 (2 heads per 128x128 block, diag 64x64 kept)
                mp = ps_alloc()
                for j in range(4):
                    for tt in range(4):
                        nc.tensor.matmul(
                            mp[:, 128 * j:128 * (j + 1)],
                            kvsb[tt][:, 1024 * g + 128 * j:1024 * g + 128 * (j + 1)],
                            kvsb[tt][:, 1024 * g + 512 + 128 * j:1024 * g + 512 + 128 * (j + 1)],
                            start=(tt == 0), stop=(tt == 3),
                        )
                Msb = mres.tile([128, 256], bf16, tag=f"Msb{g}", name=f"Msb{g}")
                for j in range(4):
                    nc.vector.tensor_copy(Msb[0:64, 64 * j:64 * j + 64],
                                          mp[0:64, 128 * j:128 * j + 64])
                    nc.vector.tensor_copy(Msb[64:128, 64 * j:64 * j + 64],
                                          mp[64:128, 128 * j + 64:128 * (j + 1)])
                bin_ = dram.tile([128, 256], bf16, name=f"bin{g}")
                bo = dram.tile([256, 256], bf16, name=f"bout{g}")
                nc.gpsimd.dma_start(bin_[0:64, :], Msb[0:64, :])
                nc.gpsimd.dma_start(bin_[64:128, :], Msb[64:128, :])
                nc.gpsimd.collective_compute(
                    "AllGather", mybir.AluOpType.bypass, replica_groups=GROUPS,
                    ins=[bin_.opt()], outs=[bo.opt()],
                )
                MrA = mres.tile([128, 256], bf16, tag=f"MrA{g}", name=f"MrA{g}")
                MrB = mres.tile([128, 256], bf16, tag=f"MrB{g}", name=f"MrB{g}")
                nc.gpsimd.dma_start(MrA[0:64, :], bo[0:64, :])
                nc.gpsimd.dma_start(MrA[64:128, :], bo[64:128, :])
                nc.gpsimd.dma_start(MrB[0:64, :], bo[128:192, :])
                nc.gpsimd.dma_start(MrB[64:128, :], bo[192:256, :])
                gath[g] = (MrA, MrB)

            def m_post(g):
                MrA, MrB = gath[g]
                for j in range(4):
                    blk = 4 * g + j
                    nc.vector.tensor_add(
                        Mbd[0:64, 128 * blk:128 * blk + 64],
                        MrA[0:64, 64 * j:64 * j + 64],
                        MrB[0:64, 64 * j:64 * j + 64])
                    nc.vector.tensor_add(
                        Mbd[64:128, 128 * blk + 64:128 * (blk + 1)],
                        MrA[64:128, 64 * j:64 * j + 64],
                        MrB[64:128, 64 * j:64 * j + 64])

            # ---- phase schedule ----
            kv_half(0, [0, 1, 2, 3], e_outer=True)   # DMA-paced: e-outer
            m_half(0)                                 # AllGather #1 in flight
            kv_half(1, [1, 2, 3, 0], e_outer=False)  # weights resident: tt-outer
            m_half(1)                                 # AllGather #2 in flight

            # ---- q (feature-major, [1024 qf, 512 t]) ----
            for qblk in range(8):
                p = ps_alloc()
                for e in range(8):
                    nc.tensor.matmul(
                        p[:],
                        wq[e][:, 128 * qblk:128 * (qblk + 1)],
                        xsb[:, 512 * e:512 * (e + 1)],
                        start=(e == 0), stop=(e == 7),
                    )
                evict(qsb[qblk][:], p[:])

            m_post(0)
            m_post(1)

            # ---- att: attT_blk = Mbd_blk.T @ qT_blk (in-place into q tiles) ----
            for blk in range(8):
                p = ps_alloc()
                nc.tensor.matmul(p[:], Mbd[:, 128 * blk:128 * (blk + 1)],
                                 qsb[blk][:], start=True, stop=True)
                evict(qsb[blk][:], p[:])
            attsb = qsb

            # ---- out = attT.T @ wpT  ((512t, 1024o)) ----
            i = 0
            for tt in range(4):
                for oc in range(2):
                    p = ps_alloc()
                    for f in range(8):
                        nc.tensor.matmul(
                            p[:],
                            attsb[f][:, 128 * tt:128 * (tt + 1)],
                            wp[f][:, 512 * oc:512 * (oc + 1)],
                            start=(f == 0), stop=(f == 7),
                        )
                    ot = op.tile([128, 512], f32, tag="osb")
                    evict(ot[:], p[:])
                    eng = nc.sync if i % 2 else nc.gpsimd
                    i += 1
                    eng.dma_start(
                        out[128 * tt:128 * (tt + 1), 512 * oc:512 * (oc + 1)],
                        ot[:],
                    )

    nc.compile()
    _built = nc
    return _built


LAST_RESULTS = None  # BassKernelResults of the most recent kernel() call


def kernel(x: np.ndarray, W_qkv: np.ndarray, W_proj: np.ndarray) -> np.ndarray:
    global LAST_RESULTS
    from concourse import bass_utils

    nc = _build()

    x = np.ascontiguousarray(x, dtype=np.float32)
    W_qkv = np.ascontiguousarray(W_qkv, dtype=np.float32)
    W_proj = np.ascontiguousarray(W_proj, dtype=np.float32)

    # head-grouping permutation: grouped feature h*64+j <- original row j*16+h
    perm = np.arange(E).reshape(HD, NH).T.ravel()
    Wq_g = W_qkv[perm]
    Wk_g = W_qkv[E + perm] * np.float32(HD ** -0.5)  # exact: 1/8
    Wv_g = W_qkv[2 * E + perm]
    # wkvT cols: [k half0 | v half0 | k half1 | v half1]
    wkvT_np = _round_bf16(np.concatenate(
        [Wk_g[0:512], Wv_g[0:512], Wk_g[512:1024], Wv_g[512:1024]], 0).T)
    wqT_np = _round_bf16(Wq_g.T)
    wpT_np = _round_bf16(W_proj.T)

    in_maps = []
    for c in range(N_CORES):
        b, half = c // 2, c % 2
        xT_c = x[b, half * TPC:(half + 1) * TPC, :].T  # [1024 f, 512 t]
        # xTi[p, 512e + t] = xT_c[128e + p, t]
        xTi_c = _round_bf16(
            xT_c.reshape(8, 128, TPC).transpose(1, 0, 2).reshape(128, 8 * TPC))
        in_maps.append({"xTi": xTi_c, "wkvT": wkvT_np, "wqT": wqT_np,
                        "wpT": wpT_np})

    import os as _os
    _tc = _os.environ.get("KERNEL_TRACE_CORES")
    _kw = {"trace_cores": [int(v) for v in _tc.split(",")]} if _tc else {}
    res = bass_utils.run_bass_kernel_spmd(nc, in_maps, core_ids=list(range(N_CORES)), **_kw)
    LAST_RESULTS = res

    outv = np.empty((B, T, E), dtype=np.float32)
    for c in range(N_CORES):
        b, half = c // 2, c % 2
        outv[b, half * TPC:(half + 1) * TPC, :] = res.results[c]["out"]
    return outv
